# revision 8
# baseline (speedup 1.0000x reference)
"""Trainium2 Bass kernel for nn_CrossAttention (B=2, Lq=Lkv=2048, E=1024, H=16, D=64).

Sharding: tensor-parallel over heads. Each of the 8 cores owns 2 heads
(a 128-wide slice of the QKV projection output and the matching 128
columns of Wo); the row-parallel Wo all-reduce is a host-side sum of
the 8 bf16 partial outputs.

Per core, single pass with all pools live so phases overlap:

  P(b,cg): per 512-token column group: x_q/x_kv loaded bf16 per-ec so
        matmuls start as data arrives (x_kv first - it feeds K and V).
        Q^T/K^T projections (8 e-chunks, bf16, biases via DVE
        tensor_scalar_add on the PSUM->SBUF copy); V built directly in
        [kv, j] layout by flipping the matmul (stationary x^T chunk,
        moving Wv) - no transpose phase. V stored [h0 | 1 | h1] with a
        shared ones column so each context matmul also emits the
        softmax denominator.
  A-part1(b,qt): scores^T = K^T.T @ Q^T with 64-row PE tiling (both
        heads concurrent); one Exp per kv chunk (scale=1/8,
        per-partition mask bias) straight from PSUM to bf16 SBUF; h0
        context accumulated inline (64-row tile pairs); h0 divide via
        reciprocal_approx_fast on the denominator row + K=1 broadcast
        matmul + one DVE multiply, landing directly in ctx^T 0:64.
  A-part2(b,qt): h1 context deferred over the buffered exp tiles
        (halves ctx PSUM), divide, SBUF->SBUF DMA shift into ctx^T
        64:128; then O(b,qt): out^T partial = Wo_c^T.T @ ctx^T, bf16
        copy, DMA out. part1(qt+1) is emitted BEFORE part2(qt) so the
        scheduler keeps the exp chain fed; part2/O/P(b1) work fills
        the exp-bound PE idle windows.

  Host sums the 8 partial outputs and adds bo + Wo @ bv (bv commutes
  through attention as a constant).
"""

import sys

if "/opt/trn_rl_repo" not in sys.path:
    sys.path.insert(0, "/opt/trn_rl_repo")

import numpy as np
import ml_dtypes

import concourse.tile as tile
from concourse import bacc, mybir
from concourse.bass_utils import run_bass_kernel_spmd

F32 = mybir.dt.float32
F32R = mybir.dt.float32r
BF16 = mybir.dt.bfloat16
AF = mybir.ActivationFunctionType

N_CORES = 8
B, LQ, LKV, E, H, D = 2, 2048, 2048, 1024, 16, 64
HC = H // N_CORES  # heads per core = 2
JC = HC * D  # feature slice per core = 128
T = B * LQ  # 4096 tokens
NEC = E // 128  # 8 e-chunks
NQT = LQ // 512  # 4 q tiles per batch
NKT = LKV // 128  # 16 kv chunks per batch
NOC = E // 128  # 8 output chunks
NCG = LQ // 512  # 4 projection column groups per batch

_NC_CACHE = {}


def build(reps=None):
    key = reps or 0
    if key in _NC_CACHE:
        return _NC_CACHE[key]
    nc = bacc.Bacc("TRN2", target_bir_lowering=False, debug=False, num_devices=N_CORES)

    xqT = nc.dram_tensor("xqT", [E, T], BF16, kind="ExternalInput").ap()
    xkT = nc.dram_tensor("xkT", [E, T], BF16, kind="ExternalInput").ap()
    wqT = nc.dram_tensor("wqT", [E, JC], BF16, kind="ExternalInput").ap()
    wkT = nc.dram_tensor("wkT", [E, JC], BF16, kind="ExternalInput").ap()
    wvT = nc.dram_tensor("wvT", [E, JC], BF16, kind="ExternalInput").ap()
    woT = nc.dram_tensor("woT", [JC, E], BF16, kind="ExternalInput").ap()
    bqd = nc.dram_tensor("bq", [JC, 1], F32, kind="ExternalInput").ap()
    bkd = nc.dram_tensor("bk", [JC, 1], F32, kind="ExternalInput").ap()
    mbd = nc.dram_tensor("mb", [B, NKT, 128], F32, kind="ExternalInput").ap()
    outT = nc.dram_tensor("outT", [E, T], BF16, kind="ExternalOutput").ap()

    from contextlib import nullcontext

    with tile.TileContext(nc) as tc, nc.allow_low_precision(reason="bf16 attention"):
        with tc.For_i(0, reps, 1) if reps else nullcontext():
         with (
             tc.tile_pool(name="const", bufs=1) as const,
             tc.tile_pool(name="big", bufs=1) as big,
             tc.tile_pool(name="xin", bufs=3) as xin,
             tc.tile_pool(name="expm", bufs=36) as expm,
             tc.tile_pool(name="dv", bufs=2) as dv,
             tc.tile_pool(name="outsb", bufs=3) as outsb,
             tc.tile_pool(name="psc", bufs=2, space="PSUM") as psc,
             tc.tile_pool(name="pcx", bufs=2, space="PSUM") as pcx,
             tc.tile_pool(name="pmm", bufs=2, space="PSUM") as pmm,
         ):
            # ---- persistent SBUF state ----
            wq_sb = const.tile([128, NEC, JC], BF16, tag="wq")
            nc.sync.dma_start(out=wq_sb, in_=wqT.rearrange("(ec p) j -> p ec j", p=128))
            wk_sb = const.tile([128, NEC, JC], BF16, tag="wk")
            nc.sync.dma_start(out=wk_sb, in_=wkT.rearrange("(ec p) j -> p ec j", p=128))
            wv_sb = const.tile([128, NEC, JC], BF16, tag="wv")
            nc.sync.dma_start(out=wv_sb, in_=wvT.rearrange("(ec p) j -> p ec j", p=128))
            wo_sb = const.tile([128, NOC, 128], BF16, tag="wo")
            nc.sync.dma_start(out=wo_sb, in_=woT.rearrange("p (oc o) -> p oc o", oc=NOC))
            bq_sb = const.tile([128, 1], F32, tag="bq")
            nc.sync.dma_start(out=bq_sb, in_=bqd)
            bk_sb = const.tile([128, 1], F32, tag="bk")
            nc.sync.dma_start(out=bk_sb, in_=bkd)
            mb_sb = const.tile([128, B, NKT], F32, tag="mb")
            nc.sync.dma_start(out=mb_sb, in_=mbd.rearrange("b kc p -> p b kc"))
            ones_f = const.tile([128, 65], F32, tag="onesf")
            nc.vector.memset(ones_f, 1.0)
            onesr = const.tile([128, 65], F32R, tag="onesr")
            nc.vector.tensor_copy(onesr, ones_f)

            qt_sb = big.tile([128, T], BF16, tag="qt")
            kt_sb = big.tile([128, T], BF16, tag="kt")
            # V as [kv, gc, [h0 d | 1 | h1 d]]; shared ones column at 64
            v_sb = big.tile([128, B * NKT, 129], BF16, tag="v")
            nc.vector.memset(v_sb[:, :, 64:65], 1.0)
            ctx_sb = big.tile([128, B * NQT, 512], BF16, tag="ctx")

            def phase_p(b, cg):
                """Load + project one 512-token column group of batch b."""
                c0 = b * LQ + cg * 512
                xk_t = xin.tile([128, NEC, 512], BF16, tag="xk", name=f"xk_{b}_{cg}")
                xq_t = xin.tile([128, NEC, 512], BF16, tag="xq", name=f"xq_{b}_{cg}")
                # x_kv first: it feeds both K (scores) and V (context)
                for ec in range(NEC):
                    nc.sync.dma_start(
                        out=xk_t[:, ec, :],
                        in_=xkT[ec * 128 : (ec + 1) * 128, c0 : c0 + 512],
                    )
                for ec in range(NEC):
                    nc.sync.dma_start(
                        out=xq_t[:, ec, :],
                        in_=xqT[ec * 128 : (ec + 1) * 128, c0 : c0 + 512],
                    )
                for w_sb, bias, dst, xt in (
                    (wk_sb, bk_sb, kt_sb, xk_t),
                    (wq_sb, bq_sb, qt_sb, xq_t),
                ):
                    pt = pmm.tile([128, 512], F32, tag="mm", name=f"p_{b}_{cg}_{dst.name}")
                    for ec in range(NEC):
                        nc.tensor.matmul(
                            pt,
                            w_sb[:, ec, :],
                            xt[:, ec, :],
                            start=(ec == 0),
                            stop=(ec == NEC - 1),
                        )
                    nc.vector.tensor_scalar_add(dst[:, c0 : c0 + 512], pt, bias)
                # V direct in [kv, j] layout: stationary x^T chunk, moving Wv
                for k4 in range(4):
                    gc = b * NKT + cg * 4 + k4
                    pv = pmm.tile([128, 128], F32, tag="mm", name=f"pv_{gc}")
                    for ec in range(NEC):
                        nc.tensor.matmul(
                            pv,
                            xk_t[:, ec, k4 * 128 : (k4 + 1) * 128],
                            wv_sb[:, ec, :],
                            start=(ec == 0),
                            stop=(ec == NEC - 1),
                        )
                    nc.vector.tensor_copy(v_sb[:, gc, 0:64], pv[:, 0:64])
                    nc.vector.tensor_copy(v_sb[:, gc, 65:129], pv[:, 64:128])

            def div_head(cxa, cxb, den_row, ctx_rows, dst_part0, ti, b, qt, h):
                """Sum the two kv-half accumulators, normalize, store ctx^T."""
                s1 = dv.tile([65, 512], F32, tag="s1", name=f"s1_{b}_{qt}_{h}")
                nc.vector.tensor_copy(s1, cxa)
                s = dv.tile([65, 512], F32, tag="s", name=f"s_{b}_{qt}_{h}")
                nc.vector.tensor_add(s, s1, cxb)
                rcp = dv.tile([65, 512], F32, tag="rcp", name=f"rcp_{b}_{qt}_{h}")
                nc.vector.reciprocal(
                    rcp[den_row : den_row + 1, :], s[den_row : den_row + 1, :]
                )
                rcpr = dv.tile([65, 512], F32R, tag="rcpr", name=f"rcpr_{b}_{qt}_{h}")
                nc.vector.tensor_copy(
                    rcpr[den_row : den_row + 1, :], rcp[den_row : den_row + 1, :]
                )
                bct = pmm.tile([65, 512], F32, tag="mm", name=f"bct_{b}_{qt}_{h}")
                nc.tensor.matmul(
                    bct,
                    onesr[den_row : den_row + 1, 0:65],
                    rcpr[den_row : den_row + 1, :],
                    start=True,
                    stop=True,
                )
                r0, r1 = ctx_rows
                if dst_part0 == 0:
                    nc.vector.tensor_mul(
                        ctx_sb[0:64, ti, :], s[r0:r1, :], bct[r0:r1, :]
                    )
                else:
                    cs = dv.tile([65, 512], BF16, tag="cs", name=f"cs_{b}_{qt}_{h}")
                    nc.vector.tensor_mul(cs, s, bct)
                    nc.sync.dma_start(out=ctx_sb[64:128, ti, :], in_=cs[r0:r1, :])

            def a_part1(b, qt):
                """Scores + exp + inline h0 context + h0 divide."""
                ti = b * NQT + qt
                q0 = b * LQ + qt * 512
                cx0a = pcx.tile([65, 512], F32, tag="cx", name=f"cx0a_{b}_{qt}")
                cx0b = pcx.tile([65, 512], F32, tag="cx", name=f"cx0b_{b}_{qt}")
                emts = []
                for kt in range(NKT):
                    k0 = b * LKV + kt * 128
                    gc = b * NKT + kt
                    sct = psc.tile([128, 2, 512], F32, tag="sc", name=f"sc_{b}_{qt}_{kt}")
                    nc.tensor.matmul(
                        sct[:, 0, :],
                        kt_sb[0:64, k0 : k0 + 128],
                        qt_sb[0:64, q0 : q0 + 512],
                        start=True,
                        stop=True,
                    )
                    nc.tensor.matmul(
                        sct[:, 1, :],
                        kt_sb[64:128, k0 : k0 + 128],
                        qt_sb[64:128, q0 : q0 + 512],
                        start=True,
                        stop=True,
                    )
                    emt = expm.tile(
                        [128, 2, 512], BF16, tag="emt", name=f"emt_{b}_{qt}_{kt}"
                    )
                    nc.scalar.activation(
                        out=emt.rearrange("p a t -> p (a t)"),
                        in_=sct.rearrange("p a t -> p (a t)"),
                        func=AF.Exp,
                        bias=mb_sb[:, b, kt : kt + 1],
                        scale=0.125,
                    )
                    emts.append(emt)
                    st, sp = (kt == 0), (kt == NKT - 1)
                    # h0 context inline: 64-row tile pair, kv halves concurrent
                    nc.tensor.matmul(
                        cx0a, v_sb[0:64, gc, 0:65], emt[0:64, 0, :], start=st, stop=sp
                    )
                    nc.tensor.matmul(
                        cx0b, v_sb[64:128, gc, 0:65], emt[64:128, 0, :],
                        start=st, stop=sp,
                    )
                # h0: rows 0:64 = ctx, row 64 = den -> lands at partitions 0:64
                div_head(cx0a, cx0b, 64, (0, 64), 0, ti, b, qt, 0)
                return emts

            def a_part2(b, qt, emts):
                """Deferred h1 context + divide + output projection."""
                ti = b * NQT + qt
                t0 = b * LQ + qt * 512
                cx1a = pcx.tile([65, 512], F32, tag="cx", name=f"cx1a_{b}_{qt}")
                cx1b = pcx.tile([65, 512], F32, tag="cx", name=f"cx1b_{b}_{qt}")
                for kt in range(NKT):
                    gc = b * NKT + kt
                    st, sp = (kt == 0), (kt == NKT - 1)
                    nc.tensor.matmul(
                        cx1a, v_sb[0:64, gc, 64:129], emts[kt][0:64, 1, :],
                        start=st, stop=sp,
                    )
                    nc.tensor.matmul(
                        cx1b, v_sb[64:128, gc, 64:129], emts[kt][64:128, 1, :],
                        start=st, stop=sp,
                    )
                # h1: row 0 = den, rows 1:65 = ctx -> shift to partitions 64:128
                div_head(cx1a, cx1b, 0, (1, 65), 64, ti, b, qt, 1)
                for oc in range(NOC):
                    opt = pmm.tile([128, 512], F32, tag="mm", name=f"o_{b}_{qt}_{oc}")
                    nc.tensor.matmul(
                        opt, wo_sb[:, oc, :], ctx_sb[:, ti, :], start=True, stop=True
                    )
                    ob = outsb.tile([128, 512], BF16, tag="ob", name=f"ob_{b}_{qt}_{oc}")
                    nc.vector.tensor_copy(ob, opt)
                    nc.sync.dma_start(
                        out=outT[oc * 128 : (oc + 1) * 128, t0 : t0 + 512], in_=ob
                    )

            # ---- schedule: part1(i+1) before part2(i); P(b1) fills A(b0).
            # All of P(b1) must be emitted before a_part1(1, 0): Tile deps
            # are program-order RAW, so a read emitted before its producer
            # would silently read garbage.
            for cg in range(NCG):
                phase_p(0, cg)
            prev_e = None
            for qt in range(NQT):
                e = a_part1(0, qt)
                if qt:
                    a_part2(0, qt - 1, prev_e)
                    phase_p(1, qt - 1)
                prev_e = e
            a_part2(0, NQT - 1, prev_e)
            phase_p(1, NQT - 1)
            for qt in range(NQT):
                e = a_part1(1, qt)
                if qt:
                    a_part2(1, qt - 1, prev_e)
                prev_e = e
            a_part2(1, NQT - 1, prev_e)

    nc.compile()
    _NC_CACHE[key] = nc
    return nc


def make_in_maps(query, key_value, mask, Wq, bq, Wk, bk, Wv, bv, Wo, bo):
    bf = ml_dtypes.bfloat16
    xqT = np.ascontiguousarray(query.reshape(T, E).T).astype(bf)
    xkT = np.ascontiguousarray(key_value.reshape(T, E).T).astype(bf)
    mb = np.where(mask != 0, 0.0, -1.0e5).astype(np.float32).reshape(B, NKT, 128)
    in_maps = []
    for c in range(N_CORES):
        sl = slice(c * JC, (c + 1) * JC)
        in_maps.append(
            {
                "xqT": xqT,
                "xkT": xkT,
                "wqT": np.ascontiguousarray(Wq[sl, :].T).astype(bf),
                "wkT": np.ascontiguousarray(Wk[sl, :].T).astype(bf),
                "wvT": np.ascontiguousarray(Wv[sl, :].T).astype(bf),
                "woT": np.ascontiguousarray(Wo[:, sl].T).astype(bf),
                "bq": bq[sl].reshape(JC, 1).astype(np.float32),
                "bk": bk[sl].reshape(JC, 1).astype(np.float32),
                "mb": mb,
            }
        )
    return in_maps


def kernel(query, key_value, mask, Wq, bq, Wk, bk, Wv, bv, Wo, bo):
    query = np.asarray(query)
    key_value = np.asarray(key_value)
    mask = np.asarray(mask)
    Wq, bq = np.asarray(Wq), np.asarray(bq)
    Wk, bk = np.asarray(Wk), np.asarray(bk)
    Wv, bv = np.asarray(Wv), np.asarray(bv)
    Wo, bo = np.asarray(Wo), np.asarray(bo)
    nc = build()
    in_maps = make_in_maps(query, key_value, mask, Wq, bq, Wk, bk, Wv, bv, Wo, bo)
    res = run_bass_kernel_spmd(nc, in_maps, list(range(N_CORES)))
    acc = np.zeros((E, T), np.float32)
    for c in range(N_CORES):
        acc += res.results[c]["outT"].astype(np.float32)
    out = np.ascontiguousarray(acc.T).reshape(B, LQ, E)
    # bv folds through attention as a constant: out += Wo @ bv; plus bo
    out += (Wo.astype(np.float64) @ bv.astype(np.float64) + bo.astype(np.float64)).astype(
        np.float32
    )
    return out.astype(np.float32)
